# revision 1
# baseline (speedup 1.0000x reference)
"""Causal self-attention (B=4, T=2048, C=1024, H=16) on 8 TRN2 NeuronCores.

Sharding: tensor-parallel pairs. Core c handles batch b = c//2 and head-half
j = c%2 (8 of the 16 heads). Each core computes the QKV projection for its
heads, causal attention, and the out-projection contracted over its half of
the features, producing a partial output. The pair-sum (the "all-reduce after
out_proj" of the tensor-parallel scheme) happens at unshard time on the host.

Structure: one fused loop — the QKV projection for token chunk n+1 is emitted
interleaved with attention for chunk n, so the Tile scheduler fills the
ACT(exp)-bound attention phase with projection matmuls and the PE never idles
long enough for the HAM clock gate to re-throttle. All matmul operands are
bf16; softmax runs in fp32 out of PSUM with the 1/8 scale folded into the ACT
free affine; the causal mask is applied post-exp on GPSIMD (fill=0); the
denominator comes free from a ones-column appended to V so the AV matmul
accumulates sum(exp) in PSUM. Weights are loaded as per-m-tile DMAs (the
first projection chain is gated on ~1.25MB instead of 4MB) and outputs are
staged into one [128, 8, 512] tile per token chunk for a single store DMA.
"""
import ml_dtypes
import numpy as np
from contextlib import ExitStack

import concourse.bass as bass
from concourse import bacc
import concourse.mybir as mybir
import concourse.tile as tile
from concourse.bass_utils import run_bass_kernel_spmd

B, T, C, H, D = 4, 2048, 1024, 16, 64
NCORES = 8
HPC = H // 2          # heads per core
F = HPC * D           # 512 features per core (per q/k/v)
KI = C // 128         # 8 contraction tiles over C
NT = T // 512         # 4 token chunks
F32 = mybir.dt.float32
BF16 = mybir.dt.bfloat16

_NC_CACHE = None


def _build():
    nc = bacc.Bacc("TRN2", target_bir_lowering=False, debug=False)
    # host-reorganized layouts (see kernel()):
    #   xr    [128, KI, T]   x[b].T ki-blocked
    #   wqm   [8, 128, KI*128]  q/k weight m-tiles, ki-blocked
    #   wv    [128, KI, F]   v weights, ki-blocked
    #   wot   [128, 4, C]    out-proj weights, ki-blocked
    xr = nc.dram_tensor("xr", [NT, 128, KI * 512], BF16, kind="ExternalInput").ap()
    wqm = nc.dram_tensor("wqm", [8, 128, KI * 128], BF16, kind="ExternalInput").ap()
    wv = nc.dram_tensor("wv", [128, KI, F], BF16, kind="ExternalInput").ap()
    wot = nc.dram_tensor("wot", [128, 4, C], BF16, kind="ExternalInput").ap()
    out = nc.dram_tensor("out", [C, T], F32, kind="ExternalOutput").ap()

    with ExitStack() as ctx:
        tc = ctx.enter_context(tile.TileContext(nc))

        # persistent SBUF tensors
        qk = ctx.enter_context(tc.tile_pool(name="qk", bufs=1))
        vp = ctx.enter_context(tc.tile_pool(name="vp", bufs=1))
        wqp = ctx.enter_context(tc.tile_pool(name="wqp", bufs=1))
        # qT/kT [128f, T] feature-major (2 heads per tile); vT token-major,
        # 8 head-groups of 65 cols (64 v features + ones col), tail-padded so
        # every 128-col weight window stays in bounds; pad/ones cols only
        # ever feed psum partitions >= 65 which are never read.
        qts = [qk.tile([128, T], BF16, tag=f"q{m}", name=f"q{m}") for m in range(4)]
        kts = [qk.tile([128, T], BF16, tag=f"k{m}", name=f"k{m}") for m in range(4)]
        vts = [vp.tile([128, 583], BF16, tag=f"v{tm}", name=f"v{tm}")
               for tm in range(T // 128)]
        wqmt = [wqp.tile([128, KI, 128], BF16, tag=f"w{m}", name=f"w{m}")
                for m in range(8)]
        wvt = wqp.tile([128, KI, F], BF16, tag="wv", name="wv")
        wost = wqp.tile([128, 4, C], BF16, tag="wo", name="wo")

        # working pools
        xp = ctx.enter_context(tc.tile_pool(name="xp", bufs=2))
        pbp = ctx.enter_context(tc.tile_pool(name="pbp", bufs=16))
        yp = ctx.enter_context(tc.tile_pool(name="yp", bufs=4))
        bp = ctx.enter_context(tc.tile_pool(name="bp", bufs=2))
        cop = ctx.enter_context(tc.tile_pool(name="cop", bufs=2))
        # PSUM: 2 banks shared matmul chains (qkv + out-proj), 4 banks scores
        # (double-buffered 2-bank tiles), 2 banks AV accumulators = 8 banks.
        mmp = ctx.enter_context(tc.tile_pool(name="mmp", bufs=2, space="PSUM"))
        scp = ctx.enter_context(tc.tile_pool(name="scp", bufs=2, space="PSUM"))
        avp = ctx.enter_context(tc.tile_pool(name="avp", bufs=1, space="PSUM"))

        fill0 = nc.gpsimd.to_reg(0.0)

        def load_x(n):
            t = xp.tile([128, KI, 512], BF16, tag="xct", name="xct")
            nc.sync.dma_start(out=t[:], in_=xr[n].rearrange(
                "p (ki t) -> p ki t", ki=KI))
            return t

        # first chain (m=4, k0) is gated on wqm[4]+xc0: issue those first
        nc.sync.dma_start(out=wqmt[4][:], in_=wqm[4].rearrange(
            "p (ki c) -> p ki c", ki=KI))
        xcs0 = load_x(0)
        for m in (0, 5, 1, 6, 2, 7, 3):
            nc.sync.dma_start(out=wqmt[m][:], in_=wqm[m].rearrange(
                "p (ki c) -> p ki c", ki=KI))
        nc.sync.dma_start(out=wvt[:], in_=wv[:])
        xcs_next = load_x(1)
        nc.sync.dma_start(out=wost[:], in_=wot[:])
        for tm in range(T // 128):
            nc.gpsimd.memset(vts[tm][:], 1.0)

        def qkv_chains(n, xct, ms):
            # ms: which of the 12 accumulation chains to emit now
            # (0..7 = q/k feature tiles, 8..11 = v token tiles)
            for m in ms:
                p = mmp.tile([128, 512], F32, tag="mmp", name="mmp")
                if m < 8:
                    for ki in range(KI):
                        nc.tensor.matmul(p[:], wqmt[m][:, ki, :], xct[:, ki, :],
                                         start=(ki == 0), stop=(ki == KI - 1))
                    dst = (qts[m] if m < 4 else kts[m - 4])[:, n * 512:(n + 1) * 512]
                    nc.vector.tensor_copy(dst, p[:])
                else:
                    tmi = m - 8
                    for ki in range(KI):
                        nc.tensor.matmul(p[:],
                                         xct[:, ki, tmi * 128:(tmi + 1) * 128],
                                         wvt[:, ki, :],
                                         start=(ki == 0), stop=(ki == KI - 1))
                    vdst = vts[n * 4 + tmi][:, 0:520].rearrange(
                        "p (h c) -> p h c", c=65)
                    nc.vector.tensor_copy(
                        vdst[:, :, 0:64],
                        p[:].rearrange("p (h c) -> p h c", c=64))

        # chunk 0: k-first chain order so attention(0) unblocks early
        xtiles = {0: xcs0, 1: xcs_next}
        qkv_chains(0, xcs0, [4, 0, 5, 1, 6, 2, 7, 3, 8, 9, 10, 11])

        def out_proj(qc, yts):
            # 2 store DMAs per chunk so the first half streams out early.
            # For the last chunk the scores pool is idle: borrow its banks so
            # all 8 chains pipeline, and split copies across ACT and DVE.
            oo = cop.tile([128, 8, 512], F32, tag="oo", name="oo")
            sct = None
            for m in range(8):
                if qc == 3 and m % 4 >= 2:
                    if m % 4 == 2:
                        sct = scp.tile([128, 2, 512], F32, tag="ps", name="ps")
                    po = sct[:, m % 4 - 2, :]
                else:
                    po = mmp.tile([128, 512], F32, tag="mmp", name="mmp")[:]
                for ki in range(4):
                    nc.tensor.matmul(po, wost[:, ki, m * 128:(m + 1) * 128],
                                     yts[ki][:],
                                     start=(ki == 0), stop=(ki == 3))
                if qc == 3 and m % 2 == 0:
                    nc.scalar.copy(oo[:, m, :], po)      # ACT is idle by then
                else:
                    nc.vector.tensor_copy(oo[:, m, :], po)
                if m == 3 or m == 7:
                    g = m // 4
                    nc.sync.dma_start(
                        out=out[g * 512:(g + 1) * 512,
                                qc * 512:(qc + 1) * 512].rearrange(
                            "(m p) t -> p m t", p=128),
                        in_=oo[:, g * 4:(g + 1) * 4, :])

        # per-(qc, hp) filler plan: (chunk, chain-ids) of projection work;
        # out-projections for earlier chunks are emitted inside later
        # (ACT-bound) windows — see below
        HP_FILLERS = {
            0: {hp: [(1, [3 * hp, 3 * hp + 1, 3 * hp + 2])] for hp in range(4)},
            1: {hp: [(2, [3 * hp, 3 * hp + 1, 3 * hp + 2])] for hp in range(4)},
            2: {hp: [(3, [3 * hp, 3 * hp + 1, 3 * hp + 2])] for hp in range(4)},
            3: {hp: [] for hp in range(4)},
        }
        yts_hist = []
        for qc in range(NT):
            n_kt = qc * 4 + 4
            if qc in (0, 1):
                xtiles[qc + 2] = load_x(qc + 2)
            yts = [yp.tile([128, 512], BF16, tag=f"y{i}", name=f"y{i}")
                   for i in range(4)]
            for hp in range(HPC // 2):       # head pairs (2*hp, 2*hp+1)
                qpair = qts[hp][:, qc * 512:(qc + 1) * 512]
                pyA = avp.tile([128, 512], F32, tag="pyA", name="pyA")
                pyB = avp.tile([128, 512], F32, tag="pyB", name="pyB")
                a0 = 2 * hp * 65
                for kt in range(n_kt):
                    ksl = kts[hp][:, kt * 128:(kt + 1) * 128]
                    d = kt - qc * 4          # diagonal block index
                    lo = max(d, 0) * 128     # cols < lo fully masked out
                    ps = scp.tile([128, 2, 512], F32, tag="ps", name="ps")
                    nc.tensor.matmul(ps[:, 0, lo:512], ksl[0:64, :],
                                     qpair[0:64, lo:512],
                                     start=True, stop=True, tile_position=(0, 0))
                    nc.tensor.matmul(ps[:, 1, lo:512], ksl[64:128, :],
                                     qpair[64:128, lo:512],
                                     start=True, stop=True, tile_position=(64, 0))
                    pb = pbp.tile([128, 2, 512], BF16, tag="pb", name="pb")
                    nc.scalar.activation(pb[:, :, lo:512], ps[:, :, lo:512],
                                         mybir.ActivationFunctionType.Exp,
                                         scale=0.125)
                    if d >= 0:
                        # zero probs where local query j < key partition i
                        nc.gpsimd.affine_select(
                            out=pb[:, :, lo:512], in_=pb[:, :, lo:512],
                            compare_op=mybir.AluOpType.is_ge, fill=fill0,
                            base=0, pattern=[[0, 2], [1, 512 - lo]],
                            channel_multiplier=-1)
                    nc.tensor.matmul(pyA[:, lo:512], vts[kt][:, a0:a0 + 128],
                                     pb[:, 0, lo:512],
                                     start=(kt == 0), stop=(kt == n_kt - 1))
                    nc.tensor.matmul(pyB[:, lo:512], vts[kt][:, a0 + 65:a0 + 193],
                                     pb[:, 1, lo:512],
                                     start=(kt == 0), stop=(kt == n_kt - 1))
                for hh, py in ((0, pyA), (1, pyB)):
                    # row 64 of py is sum(exp); normalize y = py[0:64]/py[64]
                    s1 = bp.tile([1, 512], F32, tag="s1", name="s1")
                    nc.vector.tensor_copy(s1[:], py[64:65, :])
                    r = bp.tile([1, 512], F32, tag="r", name="r")
                    nc.vector.reciprocal_approx_fast(out=r[:], in_=s1[:])
                    rb = bp.tile([64, 512], F32, tag="rb", name="rb")
                    nc.gpsimd.partition_broadcast(rb[:], r[:])
                    half = hh * 64
                    nc.vector.tensor_mul(yts[hp][half:half + 64, :],
                                         py[0:64, :], rb[:])
                # interleave filler work between head pairs
                for fn, fms in HP_FILLERS[qc][hp]:
                    qkv_chains(fn, xtiles[fn], fms)
                if qc == 2 and hp == 1:
                    out_proj(0, yts_hist[0])
                if qc == 3 and hp == 1:
                    out_proj(1, yts_hist[1])
                if qc == 3 and hp == 2:
                    out_proj(2, yts_hist[2])
            yts_hist.append(yts)
        out_proj(3, yts_hist[3])
    nc.finalize()
    return nc


def _get_nc():
    global _NC_CACHE
    if _NC_CACHE is None:
        _NC_CACHE = _build()
    return _NC_CACHE


def kernel(x, w_qkv, w_out):
    x = np.ascontiguousarray(np.asarray(x), dtype=np.float32)
    w_qkv = np.asarray(w_qkv, dtype=np.float32)
    w_out = np.asarray(w_out, dtype=np.float32)
    nc = _get_nc()

    in_maps = []
    for c in range(NCORES):
        b, j = divmod(c, 2)
        rows = np.r_[j * F:(j + 1) * F,
                     C + j * F:C + (j + 1) * F,
                     2 * C + j * F:2 * C + (j + 1) * F]
        wqkvt = w_qkv[rows, :].T.astype(ml_dtypes.bfloat16)   # [C, 3F]
        wq3 = wqkvt.reshape(KI, 128, 3 * F)
        # q/k m-tiles: wqm[m][p, ki*128+c] = wqkvt[ki*128+p, m*128+c]
        wqm = np.stack([
            np.ascontiguousarray(
                wq3[:, :, m * 128:(m + 1) * 128].transpose(1, 0, 2).reshape(
                    128, KI * 128))
            for m in range(8)])
        wv = np.ascontiguousarray(
            wq3[:, :, 2 * F:3 * F].transpose(1, 0, 2))        # [128, KI, F]
        woutt = w_out[:, j * F:(j + 1) * F].T.astype(ml_dtypes.bfloat16)  # [F, C]
        wot = np.ascontiguousarray(
            woutt.reshape(4, 128, C).transpose(1, 0, 2))      # [128, 4, C]
        # [NT, 128, KI*512]: per chunk, per partition, ki-blocks contiguous
        xT = x[b].T.reshape(KI, 128, NT, 512)
        xr = np.ascontiguousarray(
            xT.transpose(2, 1, 0, 3).reshape(NT, 128, KI * 512)).astype(
                ml_dtypes.bfloat16)
        in_maps.append({"xr": xr, "wqm": wqm, "wv": wv, "wot": wot})

    res = run_bass_kernel_spmd(nc, in_maps, core_ids=list(range(NCORES)))
    y = np.empty((B, T, C), np.float32)
    for b in range(B):
        y[b] = (res.results[2 * b]["out"] + res.results[2 * b + 1]["out"]).T
    return y



# revision 8
# speedup vs baseline: 1.0248x; 1.0248x over previous
"""Causal self-attention (B=4, T=2048, C=1024, H=16) on 8 TRN2 NeuronCores.

Sharding: tensor-parallel pairs. Core c handles batch b = c//2 and head-half
j = c%2 (8 of the 16 heads). Each core computes the QKV projection for its
heads, causal attention, and the out-projection contracted over its half of
the features, producing a partial output. The pair-sum (the "all-reduce after
out_proj" of the tensor-parallel scheme) happens at unshard time on the host.

Structure: one fused loop — the QKV projection for token chunk n+1 is emitted
interleaved with attention for chunk n, so the Tile scheduler fills the
ACT(exp)-bound attention phase with projection matmuls and the PE never idles
long enough for the HAM clock gate to re-throttle. All matmul operands are
bf16; softmax runs in fp32 out of PSUM with the 1/8 scale folded into the ACT
free affine; the causal mask is applied post-exp on GPSIMD (fill=0); the
denominator comes free from a ones-column appended to V so the AV matmul
accumulates sum(exp) in PSUM.

Startup/teardown: a dummy partition_broadcast at t=0 pre-triggers the GPSIMD
custom-op library load (otherwise an ~9us mid-kernel stall before the first
real broadcast); weight m-tiles are posted as 3 batched DMAs and the first x
chunk in 2 halves so the first projection chain is gated on ~0.6MB; the less
urgent input loads post from the Scalar DGE queue in parallel. Output is
stored as fp16 (host converts/sums in fp32), halving store DMA, with 4 store
posts per token chunk so the tail only waits on the last 256 output rows.
"""
import ml_dtypes
import numpy as np
from contextlib import ExitStack

import concourse.bass as bass
from concourse import bacc
import concourse.mybir as mybir
import concourse.tile as tile
from concourse.bass_utils import run_bass_kernel_spmd

B, T, C, H, D = 4, 2048, 1024, 16, 64
NCORES = 8
HPC = H // 2          # heads per core
F = HPC * D           # 512 features per core (per q/k/v)
KI = C // 128         # 8 contraction tiles over C
NT = T // 512         # 4 token chunks
F32 = mybir.dt.float32
F16 = mybir.dt.float16
BF16 = mybir.dt.bfloat16

_NC_CACHE = None


def _build():
    nc = bacc.Bacc("TRN2", target_bir_lowering=False, debug=False)
    # host-reorganized layouts (see kernel()):
    #   xr    [128, KI, T]   x[b].T ki-blocked
    #   wqm   [8, 128, KI*128]  q/k weight m-tiles, ki-blocked
    #   wv    [128, KI, F]   v weights, ki-blocked
    #   wot   [128, 4, C]    out-proj weights, ki-blocked
    xr = nc.dram_tensor("xr", [NT, 128, KI * 512], BF16, kind="ExternalInput").ap()
    wqm = nc.dram_tensor("wqm", [8, 128, KI * 128], BF16, kind="ExternalInput").ap()
    wv = nc.dram_tensor("wv", [128, KI, F], BF16, kind="ExternalInput").ap()
    wot = nc.dram_tensor("wot", [128, 4, C], BF16, kind="ExternalInput").ap()
    # out layout [chunk, m-pair, partition, m, t]: per-partition lines are
    # 2*512 fp16 = 2KB contiguous, which the DMA engines need for full rate
    # (a [C, T] fp16 layout leaves only 1KB lines and measures ~160 GB/s).
    out = nc.dram_tensor("out", [NT, 4, 128, 2, 512], F16,
                         kind="ExternalOutput").ap()

    with ExitStack() as ctx:
        tc = ctx.enter_context(tile.TileContext(nc))

        # persistent SBUF tensors
        qk = ctx.enter_context(tc.tile_pool(name="qk", bufs=1))
        vp = ctx.enter_context(tc.tile_pool(name="vp", bufs=1))
        wqp = ctx.enter_context(tc.tile_pool(name="wqp", bufs=1))
        # qT/kT [128f, T] feature-major (2 heads per tile); vT token-major,
        # 8 head-groups of 65 cols (64 v features + ones col), tail-padded so
        # every 128-col weight window stays in bounds; pad/ones cols only
        # ever feed psum partitions >= 65 which are never read.
        qts = [qk.tile([128, T], BF16, tag=f"q{m}", name=f"q{m}") for m in range(4)]
        kts = [qk.tile([128, T], BF16, tag=f"k{m}", name=f"k{m}") for m in range(4)]
        vts = [vp.tile([128, 583], BF16, tag=f"v{tm}", name=f"v{tm}")
               for tm in range(T // 128)]
        wqmt = wqp.tile([128, 8, KI, 128], BF16, tag="wq", name="wq")
        wvt = wqp.tile([128, KI, F], BF16, tag="wv", name="wv")
        wost = wqp.tile([128, 4, C], BF16, tag="wo", name="wo")

        # working pools
        xp = ctx.enter_context(tc.tile_pool(name="xp", bufs=2))
        pbp = ctx.enter_context(tc.tile_pool(name="pbp", bufs=16))
        yp = ctx.enter_context(tc.tile_pool(name="yp", bufs=4))
        bp = ctx.enter_context(tc.tile_pool(name="bp", bufs=2))
        cop = ctx.enter_context(tc.tile_pool(name="cop", bufs=2))
        # PSUM: 2 banks shared matmul chains (qkv + out-proj), 4 banks scores
        # (double-buffered 2-bank tiles), 2 banks AV accumulators = 8 banks.
        mmp = ctx.enter_context(tc.tile_pool(name="mmp", bufs=2, space="PSUM"))
        scp = ctx.enter_context(tc.tile_pool(name="scp", bufs=2, space="PSUM"))
        avp = ctx.enter_context(tc.tile_pool(name="avp", bufs=1, space="PSUM"))

        fill0 = nc.gpsimd.to_reg(0.0)

        # dummy partition_broadcast: forces the GPSIMD custom-op library
        # (which contains it) to load now, overlapped with the input DMAs,
        # instead of stalling ~9us at the first real broadcast mid-kernel.
        pbw = bp.tile([1, 8], F32, tag="pbw", name="pbw")
        pbo = bp.tile([64, 8], F32, tag="pbo", name="pbo")
        nc.gpsimd.memset(pbw[:], 0.0)
        nc.gpsimd.partition_broadcast(pbo[:, :], pbw[0:1, :])

        def load_x(n, eng=None, split=False):
            t = xp.tile([128, KI, 512], BF16, tag="xct", name="xct")
            src = xr[n].rearrange("p (ki t) -> p ki t", ki=KI)
            e = eng or nc.sync
            if split:
                h = KI // 2
                e.dma_start(out=t[:, 0:h], in_=src[:, 0:h])
                e.dma_start(out=t[:, h:KI], in_=src[:, h:KI])
            else:
                e.dma_start(out=t[:], in_=src)
            return t

        # DMA ordering: the Scalar DGE's preamble finishes ~1us before Sync's,
        # so post the two transfers gating the first matmul chain (wqm[4] and
        # the first half of x chunk 0) there; everything else goes on the Sync
        # queue IN CONSUMPTION ORDER so the critical pieces aren't sharing
        # HBM bandwidth with bulk loads they don't need yet.
        nc.scalar.dma_start(out=wqmt[:, 4], in_=wqm[4].rearrange(
            "p (ki c) -> p ki c", ki=KI))
        xcs0 = xp.tile([128, KI, 512], BF16, tag="xct", name="xct")
        xsrc0 = xr[0].rearrange("p (ki t) -> p ki t", ki=KI)
        nc.scalar.dma_start(out=xcs0[:, 0:4], in_=xsrc0[:, 0:4])
        nc.sync.dma_start(out=xcs0[:, 4:KI], in_=xsrc0[:, 4:KI])
        nc.sync.dma_start(out=wqmt[:, 0:4], in_=wqm[0:4].rearrange(
            "m p (ki c) -> p m ki c", ki=KI))
        nc.sync.dma_start(out=wqmt[:, 5:8], in_=wqm[5:8].rearrange(
            "m p (ki c) -> p m ki c", ki=KI))
        nc.sync.dma_start(out=wvt[:], in_=wv[:])
        xcs_next = load_x(1)
        nc.sync.dma_start(out=wost[:], in_=wot[:])
        for tm in range(T // 128):
            nc.vector.memset(vts[tm][:], 1.0)

        def qkv_chains(n, xct, ms):
            # ms: which of the 12 accumulation chains to emit now
            # (0..7 = q/k feature tiles, 8..11 = v token tiles)
            for m in ms:
                p = mmp.tile([128, 512], F32, tag="mmp", name="mmp")
                if m < 8:
                    for ki in range(KI):
                        nc.tensor.matmul(p[:], wqmt[:, m, ki, :], xct[:, ki, :],
                                         start=(ki == 0), stop=(ki == KI - 1))
                    dst = (qts[m] if m < 4 else kts[m - 4])[:, n * 512:(n + 1) * 512]
                    nc.vector.tensor_copy(dst, p[:])
                else:
                    tmi = m - 8
                    for ki in range(KI):
                        nc.tensor.matmul(p[:],
                                         xct[:, ki, tmi * 128:(tmi + 1) * 128],
                                         wvt[:, ki, :],
                                         start=(ki == 0), stop=(ki == KI - 1))
                    vdst = vts[n * 4 + tmi][:, 0:520].rearrange(
                        "p (h c) -> p h c", c=65)
                    nc.vector.tensor_copy(
                        vdst[:, :, 0:64],
                        p[:].rearrange("p (h c) -> p h c", c=64))

        # chunk 0: k-first chain order so attention(0) unblocks early
        xtiles = {0: xcs0, 1: xcs_next}
        qkv_chains(0, xcs0, [4, 0, 5, 1, 6, 2, 7, 3, 8, 9, 10, 11])

        def out_proj(qc, yts):
            # 4 store DMAs per chunk so output streams out as chains finish.
            # For the last chunk the scores pool is idle: borrow its banks so
            # all 8 chains pipeline, and split copies across ACT and DVE.
            oo = cop.tile([128, 8, 512], F16, tag="oo", name="oo")
            sct = None
            for m in range(8):
                if qc == 3 and m % 4 >= 2:
                    if m % 4 == 2:
                        sct = scp.tile([128, 2, 512], F32, tag="ps", name="ps")
                    po = sct[:, m % 4 - 2, :]
                else:
                    po = mmp.tile([128, 512], F32, tag="mmp", name="mmp")[:]
                for ki in range(4):
                    nc.tensor.matmul(po, wost[:, ki, m * 128:(m + 1) * 128],
                                     yts[ki][:],
                                     start=(ki == 0), stop=(ki == 3))
                if qc == 3 and m % 2 == 0:
                    nc.scalar.copy(oo[:, m, :], po)      # ACT is idle by then
                else:
                    nc.vector.tensor_copy(oo[:, m, :], po)
                if m % 2 == 1:
                    g = m // 2
                    nc.sync.dma_start(out=out[qc, g],
                                      in_=oo[:, g * 2:(g + 1) * 2, :])

        # per-(qc, hp) filler plan: (chunk, chain-ids) of projection work;
        # out-projections for earlier chunks are emitted inside later
        # (ACT-bound) windows — see below
        HP_FILLERS = {
            0: {hp: [(1, [3 * hp, 3 * hp + 1, 3 * hp + 2])] for hp in range(4)},
            1: {hp: [(2, [3 * hp, 3 * hp + 1, 3 * hp + 2])] for hp in range(4)},
            2: {hp: [(3, [3 * hp, 3 * hp + 1, 3 * hp + 2])] for hp in range(4)},
            3: {hp: [] for hp in range(4)},
        }
        yts_hist = []
        for qc in range(NT):
            n_kt = qc * 4 + 4
            if qc in (0, 1):
                xtiles[qc + 2] = load_x(qc + 2)
            yts = [yp.tile([128, 512], BF16, tag=f"y{i}", name=f"y{i}")
                   for i in range(4)]
            for hp in range(HPC // 2):       # head pairs (2*hp, 2*hp+1)
                qpair = qts[hp][:, qc * 512:(qc + 1) * 512]
                pyA = avp.tile([128, 512], F32, tag="pyA", name="pyA")
                pyB = avp.tile([128, 512], F32, tag="pyB", name="pyB")
                a0 = 2 * hp * 65
                for kt in range(n_kt):
                    ksl = kts[hp][:, kt * 128:(kt + 1) * 128]
                    d = kt - qc * 4          # diagonal block index
                    lo = max(d, 0) * 128     # cols < lo fully masked out
                    ps = scp.tile([128, 2, 512], F32, tag="ps", name="ps")
                    nc.tensor.matmul(ps[:, 0, lo:512], ksl[0:64, :],
                                     qpair[0:64, lo:512],
                                     start=True, stop=True, tile_position=(0, 0))
                    nc.tensor.matmul(ps[:, 1, lo:512], ksl[64:128, :],
                                     qpair[64:128, lo:512],
                                     start=True, stop=True, tile_position=(64, 0))
                    pb = pbp.tile([128, 2, 512], BF16, tag="pb", name="pb")
                    nc.scalar.activation(pb[:, :, lo:512], ps[:, :, lo:512],
                                         mybir.ActivationFunctionType.Exp,
                                         scale=0.125)
                    if d >= 0:
                        # zero probs where local query j < key partition i
                        nc.gpsimd.affine_select(
                            out=pb[:, :, lo:512], in_=pb[:, :, lo:512],
                            compare_op=mybir.AluOpType.is_ge, fill=fill0,
                            base=0, pattern=[[0, 2], [1, 512 - lo]],
                            channel_multiplier=-1)
                    nc.tensor.matmul(pyA[:, lo:512], vts[kt][:, a0:a0 + 128],
                                     pb[:, 0, lo:512],
                                     start=(kt == 0), stop=(kt == n_kt - 1))
                    nc.tensor.matmul(pyB[:, lo:512], vts[kt][:, a0 + 65:a0 + 193],
                                     pb[:, 1, lo:512],
                                     start=(kt == 0), stop=(kt == n_kt - 1))
                for hh, py in ((0, pyA), (1, pyB)):
                    # row 64 of py is sum(exp); normalize y = py[0:64]/py[64]
                    # (recip is a custom DVE op: PSUM src reads garbage, so
                    # stage the denominator row through SBUF first)
                    s1 = bp.tile([1, 512], F32, tag="s1", name="s1")
                    nc.vector.tensor_copy(s1[:], py[64:65, :])
                    r = bp.tile([1, 512], F32, tag="r", name="r")
                    nc.vector.reciprocal_approx_fast(out=r[:], in_=s1[:])
                    rb = bp.tile([64, 512], F32, tag="rb", name="rb")
                    nc.gpsimd.partition_broadcast(rb[:], r[:])
                    half = hh * 64
                    nc.vector.tensor_mul(yts[hp][half:half + 64, :],
                                         py[0:64, :], rb[:])
                # interleave filler work between head pairs
                for fn, fms in HP_FILLERS[qc][hp]:
                    qkv_chains(fn, xtiles[fn], fms)
                if qc == 2 and hp == 1:
                    out_proj(0, yts_hist[0])
                if qc == 3 and hp == 1:
                    out_proj(1, yts_hist[1])
                if qc == 3 and hp == 2:
                    out_proj(2, yts_hist[2])
            yts_hist.append(yts)
        out_proj(3, yts_hist[3])
    nc.finalize()
    return nc


def _get_nc():
    global _NC_CACHE
    if _NC_CACHE is None:
        _NC_CACHE = _build()
    return _NC_CACHE


def kernel(x, w_qkv, w_out):
    x = np.ascontiguousarray(np.asarray(x), dtype=np.float32)
    w_qkv = np.asarray(w_qkv, dtype=np.float32)
    w_out = np.asarray(w_out, dtype=np.float32)
    nc = _get_nc()

    in_maps = []
    for c in range(NCORES):
        b, j = divmod(c, 2)
        rows = np.r_[j * F:(j + 1) * F,
                     C + j * F:C + (j + 1) * F,
                     2 * C + j * F:2 * C + (j + 1) * F]
        wqkvt = w_qkv[rows, :].T.astype(ml_dtypes.bfloat16)   # [C, 3F]
        wq3 = wqkvt.reshape(KI, 128, 3 * F)
        # q/k m-tiles: wqm[m][p, ki*128+c] = wqkvt[ki*128+p, m*128+c]
        wqm = np.stack([
            np.ascontiguousarray(
                wq3[:, :, m * 128:(m + 1) * 128].transpose(1, 0, 2).reshape(
                    128, KI * 128))
            for m in range(8)])
        wv = np.ascontiguousarray(
            wq3[:, :, 2 * F:3 * F].transpose(1, 0, 2))        # [128, KI, F]
        woutt = w_out[:, j * F:(j + 1) * F].T.astype(ml_dtypes.bfloat16)  # [F, C]
        wot = np.ascontiguousarray(
            woutt.reshape(4, 128, C).transpose(1, 0, 2))      # [128, 4, C]
        # [NT, 128, KI*512]: per chunk, per partition, ki-blocks contiguous
        xT = x[b].T.reshape(KI, 128, NT, 512)
        xr = np.ascontiguousarray(
            xT.transpose(2, 1, 0, 3).reshape(NT, 128, KI * 512)).astype(
                ml_dtypes.bfloat16)
        in_maps.append({"xr": xr, "wqm": wqm, "wv": wv, "wot": wot})

    res = run_bass_kernel_spmd(nc, in_maps, core_ids=list(range(NCORES)))

    def unshard(o):
        # [qc, g, p, m, t] -> [C, T]: feature c = (2g + m)*128 + p
        return np.asarray(o).astype(np.float32).transpose(
            1, 3, 2, 0, 4).reshape(C, T)

    y = np.empty((B, T, C), np.float32)
    for b in range(B):
        y[b] = (unshard(res.results[2 * b]["out"]) +
                unshard(res.results[2 * b + 1]["out"])).T
    return y


# revision 11
# speedup vs baseline: 1.0333x; 1.0083x over previous
"""Causal self-attention (B=4, T=2048, C=1024, H=16) on 8 TRN2 NeuronCores.

Sharding: tensor-parallel pairs. Core c handles batch b = c//2 and head-half
j = c%2 (8 of the 16 heads). Each core computes the QKV projection for its
heads, causal attention, and the out-projection contracted over its half of
the features, producing a partial output. The pair-sum (the "all-reduce after
out_proj" of the tensor-parallel scheme) happens at unshard time on the host.

Structure: one fused loop — the QKV projection for token chunk n+1 is emitted
interleaved with attention for chunk n, so the Tile scheduler fills the
ACT(exp)-bound attention phase with projection matmuls and the PE never idles
long enough for the HAM clock gate to re-throttle. All matmul operands are
bf16; softmax runs in fp32 out of PSUM with the 1/8 scale folded into the ACT
free affine; the causal mask is applied post-exp on GPSIMD (fill=0); the
denominator comes free from a ones-column appended to V so the AV matmul
accumulates sum(exp) in PSUM.

Pipeline details:
- a dummy partition_broadcast at t=0 pre-triggers the GPSIMD custom-op
  library load (otherwise ~9us of mid-kernel stall at the first broadcast);
- input DMAs post on the Sync DGE queue in consumption order (the Scalar DGE
  queue measures ~3x slower startup); x chunk 0 is split so the first chain
  is gated on ~0.75MB; warm-up matmuls on a memset tile spin the PE past the
  HAM clock-gate window while the first inputs stream in;
- each iteration pre-emits the next iteration's first score blocks so the
  exp stream never starves at iteration boundaries;
- the last chunk's out-projection runs ki 0-2 of every chain before the final
  normalize (borrowing the idle scores/AV PSUM banks), so only one matmul per
  chain remains after the last y tile;
- output is stored as fp16 [chunk, pair, p, m, t] (2KB DMA lines), 4 store
  posts per chunk; the host converts/sums in fp32.
"""
import ml_dtypes
import numpy as np
from contextlib import ExitStack

import concourse.bass as bass
from concourse import bacc
import concourse.mybir as mybir
import concourse.tile as tile
from concourse.bass_utils import run_bass_kernel_spmd

B, T, C, H, D = 4, 2048, 1024, 16, 64
NCORES = 8
HPC = H // 2          # heads per core
F = HPC * D           # 512 features per core (per q/k/v)
KI = C // 128         # 8 contraction tiles over C
NT = T // 512         # 4 token chunks
F32 = mybir.dt.float32
F16 = mybir.dt.float16
BF16 = mybir.dt.bfloat16

_NC_CACHE = None


def _build():
    nc = bacc.Bacc("TRN2", target_bir_lowering=False, debug=False)
    # host-reorganized layouts (see kernel()):
    #   xr    [128, KI, T]   x[b].T ki-blocked
    #   wqm   [8, 128, KI*128]  q/k weight m-tiles, ki-blocked
    #   wv    [128, KI, F]   v weights, ki-blocked
    #   wot   [128, 4, C]    out-proj weights, ki-blocked
    xr = nc.dram_tensor("xr", [NT, 128, KI * 512], BF16, kind="ExternalInput").ap()
    wqm = nc.dram_tensor("wqm", [8, 128, KI * 128], BF16, kind="ExternalInput").ap()
    wv = nc.dram_tensor("wv", [128, KI, F], BF16, kind="ExternalInput").ap()
    wot = nc.dram_tensor("wot", [128, 4, C], BF16, kind="ExternalInput").ap()
    # out layout [chunk, m-pair, partition, m, t]: per-partition lines are
    # 2*512 fp16 = 2KB contiguous, which the DMA engines need for full rate
    # (a [C, T] fp16 layout leaves only 1KB lines and measures ~160 GB/s).
    out = nc.dram_tensor("out", [NT, 4, 128, 2, 512], F16,
                         kind="ExternalOutput").ap()

    with ExitStack() as ctx:
        tc = ctx.enter_context(tile.TileContext(nc))

        # persistent SBUF tensors
        qk = ctx.enter_context(tc.tile_pool(name="qk", bufs=1))
        vp = ctx.enter_context(tc.tile_pool(name="vp", bufs=1))
        wqp = ctx.enter_context(tc.tile_pool(name="wqp", bufs=1))
        # qT/kT [128f, T] feature-major (2 heads per tile); vT token-major,
        # 8 head-groups of 65 cols (64 v features + ones col), tail-padded so
        # every 128-col weight window stays in bounds; pad/ones cols only
        # ever feed psum partitions >= 65 which are never read.
        qts = [qk.tile([128, T], BF16, tag=f"q{m}", name=f"q{m}") for m in range(4)]
        kts = [qk.tile([128, T], BF16, tag=f"k{m}", name=f"k{m}") for m in range(4)]
        vts = [vp.tile([128, 583], BF16, tag=f"v{tm}", name=f"v{tm}")
               for tm in range(T // 128)]
        wqmt = wqp.tile([128, 8, KI, 128], BF16, tag="wq", name="wq")
        wvt = wqp.tile([128, KI, F], BF16, tag="wv", name="wv")
        wost = wqp.tile([128, 4, C], BF16, tag="wo", name="wo")

        # working pools
        xp = ctx.enter_context(tc.tile_pool(name="xp", bufs=2))
        pbp = ctx.enter_context(tc.tile_pool(name="pbp", bufs=16))
        yp = ctx.enter_context(tc.tile_pool(name="yp", bufs=4))
        bp = ctx.enter_context(tc.tile_pool(name="bp", bufs=2))
        cop = ctx.enter_context(tc.tile_pool(name="cop", bufs=2))
        # PSUM: 2 banks shared matmul chains (qkv + out-proj), 4 banks scores
        # (double-buffered 2-bank tiles), 2 banks AV accumulators = 8 banks.
        mmp = ctx.enter_context(tc.tile_pool(name="mmp", bufs=2, space="PSUM"))
        scp = ctx.enter_context(tc.tile_pool(name="scp", bufs=2, space="PSUM"))
        avp = ctx.enter_context(tc.tile_pool(name="avp", bufs=1, space="PSUM"))

        fill0 = nc.gpsimd.to_reg(0.0)

        # dummy partition_broadcast: forces the GPSIMD custom-op library
        # (which contains it) to load now, overlapped with the input DMAs,
        # instead of stalling ~9us at the first real broadcast mid-kernel.
        pbw = bp.tile([1, 8], F32, tag="pbw", name="pbw")
        pbo = bp.tile([64, 8], F32, tag="pbo", name="pbo")
        nc.gpsimd.memset(pbw[:], 0.0)
        nc.gpsimd.partition_broadcast(pbo[:, :], pbw[0:1, :])

        def load_x(n):
            t = xp.tile([128, KI, 512], BF16, tag="xct", name="xct")
            nc.sync.dma_start(out=t[:], in_=xr[n].rearrange(
                "p (ki t) -> p ki t", ki=KI))
            return t

        # all input DMAs on the Sync DGE queue, in consumption order; the
        # first chain (m=4, k0) is gated on wqm[4] + the first half of x0.
        nc.sync.dma_start(out=wqmt[:, 4], in_=wqm[4].rearrange(
            "p (ki c) -> p ki c", ki=KI))
        xcs0 = xp.tile([128, KI, 512], BF16, tag="xct", name="xct")
        xsrc0 = xr[0].rearrange("p (ki t) -> p ki t", ki=KI)
        nc.sync.dma_start(out=xcs0[:, 0:4], in_=xsrc0[:, 0:4])
        nc.sync.dma_start(out=xcs0[:, 4:KI], in_=xsrc0[:, 4:KI])
        nc.sync.dma_start(out=wqmt[:, 0:4], in_=wqm[0:4].rearrange(
            "m p (ki c) -> p m ki c", ki=KI))
        nc.sync.dma_start(out=wqmt[:, 5:8], in_=wqm[5:8].rearrange(
            "m p (ki c) -> p m ki c", ki=KI))
        nc.sync.dma_start(out=wvt[:], in_=wv[:])
        xcs_next = load_x(1)
        nc.sync.dma_start(out=wost[:], in_=wot[:])
        for tm in range(T // 128):
            nc.vector.memset(vts[tm][:], 1.0)

        # HAM warm-up: ~4.5us of dummy matmuls on the first memset v tile so
        # the PE clock gate opens while the real inputs are still streaming.
        wup = scp.tile([128, 2, 512], F32, tag="ps", name="wup")
        for i in range(10):
            nc.tensor.matmul(wup[:, 0, :], vts[0][:, 0:128], vts[0][:, 0:512],
                             start=(i == 0), stop=(i == 9))

        def qkv_chains(n, xct, ms):
            # ms: which of the 12 accumulation chains to emit now
            # (0..7 = q/k feature tiles, 8..11 = v token tiles)
            for m in ms:
                p = mmp.tile([128, 512], F32, tag="mmp", name="mmp")
                if m < 8:
                    for ki in range(KI):
                        nc.tensor.matmul(p[:], wqmt[:, m, ki, :], xct[:, ki, :],
                                         start=(ki == 0), stop=(ki == KI - 1))
                    dst = (qts[m] if m < 4 else kts[m - 4])[:, n * 512:(n + 1) * 512]
                    nc.vector.tensor_copy(dst, p[:])
                else:
                    tmi = m - 8
                    for ki in range(KI):
                        nc.tensor.matmul(p[:],
                                         xct[:, ki, tmi * 128:(tmi + 1) * 128],
                                         wvt[:, ki, :],
                                         start=(ki == 0), stop=(ki == KI - 1))
                    vdst = vts[n * 4 + tmi][:, 0:520].rearrange(
                        "p (h c) -> p h c", c=65)
                    nc.vector.tensor_copy(
                        vdst[:, :, 0:64],
                        p[:].rearrange("p (h c) -> p h c", c=64))

        # chunk 0: k-first chain order so attention(0) unblocks early
        xtiles = {0: xcs0, 1: xcs_next}
        qkv_chains(0, xcs0, [4, 0, 5, 1, 6, 2, 7, 3, 8, 9, 10, 11])

        def sc_block(qc, hp, kt):
            # scores + exp + causal mask for one 128-key block; returns the
            # bf16 probability tile (and the lo offset for the AV matmuls).
            qpair = qts[hp][:, qc * 512:(qc + 1) * 512]
            ksl = kts[hp][:, kt * 128:(kt + 1) * 128]
            d = kt - qc * 4              # diagonal block index
            lo = max(d, 0) * 128         # cols < lo fully masked out
            ps = scp.tile([128, 2, 512], F32, tag="ps", name="ps")
            nc.tensor.matmul(ps[:, 0, lo:512], ksl[0:64, :],
                             qpair[0:64, lo:512],
                             start=True, stop=True, tile_position=(0, 0))
            nc.tensor.matmul(ps[:, 1, lo:512], ksl[64:128, :],
                             qpair[64:128, lo:512],
                             start=True, stop=True, tile_position=(64, 0))
            pb = pbp.tile([128, 2, 512], BF16, tag="pb", name="pb")
            nc.scalar.activation(pb[:, :, lo:512], ps[:, :, lo:512],
                                 mybir.ActivationFunctionType.Exp,
                                 scale=0.125)
            if d >= 0:
                # zero probs where local query j < key partition i
                nc.gpsimd.affine_select(
                    out=pb[:, :, lo:512], in_=pb[:, :, lo:512],
                    compare_op=mybir.AluOpType.is_ge, fill=fill0,
                    base=0, pattern=[[0, 2], [1, 512 - lo]],
                    channel_multiplier=-1)
            return pb, lo

        def out_proj(qc, yts):
            # 4 store DMAs per chunk so output streams out as chains finish
            oo = cop.tile([128, 8, 512], F16, tag="oo", name="oo")
            for m in range(8):
                po = mmp.tile([128, 512], F32, tag="mmp", name="mmp")[:]
                for ki in range(4):
                    nc.tensor.matmul(po, wost[:, ki, m * 128:(m + 1) * 128],
                                     yts[ki][:],
                                     start=(ki == 0), stop=(ki == 3))
                nc.vector.tensor_copy(oo[:, m, :], po)
                if m % 2 == 1:
                    g = m // 2
                    nc.sync.dma_start(out=out[qc, g],
                                      in_=oo[:, g * 2:(g + 1) * 2, :])

        # per-(qc, hp) filler plan: (chunk, chain-ids) of projection work;
        # out-projections for earlier chunks are emitted inside later
        # (ACT-bound) windows — see below
        HP_FILLERS = {
            0: {hp: [(1, [3 * hp, 3 * hp + 1, 3 * hp + 2])] for hp in range(4)},
            1: {hp: [(2, [3 * hp, 3 * hp + 1, 3 * hp + 2])] for hp in range(4)},
            2: {hp: [(3, [3 * hp, 3 * hp + 1, 3 * hp + 2])] for hp in range(4)},
            3: {hp: [] for hp in range(4)},
        }
        LOOKAHEAD = 3    # score blocks of iteration i+1 pre-emitted inside i
        iters = [(qc, hp) for qc in range(NT) for hp in range(HPC // 2)]
        pre_pbs = {}
        yts_hist = []
        yts = None
        for idx, (qc, hp) in enumerate(iters):
            n_kt = qc * 4 + 4
            if hp == 0:
                if qc in (0, 1):
                    xtiles[qc + 2] = load_x(qc + 2)
                yts = [yp.tile([128, 512], BF16, tag=f"y{i}", name=f"y{i}")
                       for i in range(4)]
            pyA = avp.tile([128, 512], F32, tag="pyA", name="pyA")
            pyB = avp.tile([128, 512], F32, tag="pyB", name="pyB")
            a0 = 2 * hp * 65
            pbs = pre_pbs.pop((qc, hp), [])
            for kt in range(n_kt):
                if kt < len(pbs):
                    pb, lo = pbs[kt]
                else:
                    pb, lo = sc_block(qc, hp, kt)
                nc.tensor.matmul(pyA[:, lo:512], vts[kt][:, a0:a0 + 128],
                                 pb[:, 0, lo:512],
                                 start=(kt == 0), stop=(kt == n_kt - 1))
                nc.tensor.matmul(pyB[:, lo:512], vts[kt][:, a0 + 65:a0 + 193],
                                 pb[:, 1, lo:512],
                                 start=(kt == 0), stop=(kt == n_kt - 1))
            # pre-emit the next iteration's first score blocks so the exp
            # stream has work while this iteration's normalize/fillers run
            if idx + 1 < len(iters):
                nq, nh = iters[idx + 1]
                pre_pbs[(nq, nh)] = [sc_block(nq, nh, k2)
                                     for k2 in range(min(LOOKAHEAD, nq * 4 + 4))]
            if (qc, hp) == (3, 3):
                # chunk-3 out-proj prefix: ki 0-2 of chains 0-5 into the now
                # idle mmp + scores banks, before the last normalize gates
                # everything; chains 6,7 take the AV banks right after.
                op3 = [None] * 8
                op3[0] = mmp.tile([128, 512], F32, tag="mmp", name="mmp")[:]
                op3[1] = mmp.tile([128, 512], F32, tag="mmp", name="mmp")[:]
                sct1 = scp.tile([128, 2, 512], F32, tag="ps", name="ps")
                sct2 = scp.tile([128, 2, 512], F32, tag="ps", name="ps")
                op3[2], op3[3] = sct1[:, 0, :], sct1[:, 1, :]
                op3[4], op3[5] = sct2[:, 0, :], sct2[:, 1, :]
                for m in range(6):
                    for ki in range(3):
                        nc.tensor.matmul(op3[m],
                                         wost[:, ki, m * 128:(m + 1) * 128],
                                         yts[ki][:],
                                         start=(ki == 0), stop=False)
            for hh, py in ((0, pyA), (1, pyB)):
                # row 64 of py is sum(exp); normalize y = py[0:64]/py[64]
                # (recip is a custom DVE op: PSUM src reads garbage, so
                # stage the denominator row through SBUF first)
                s1 = bp.tile([1, 512], F32, tag="s1", name="s1")
                nc.vector.tensor_copy(s1[:], py[64:65, :])
                r = bp.tile([1, 512], F32, tag="r", name="r")
                nc.vector.reciprocal_approx_fast(out=r[:], in_=s1[:])
                rb = bp.tile([64, 512], F32, tag="rb", name="rb")
                nc.gpsimd.partition_broadcast(rb[:], r[:])
                half = hh * 64
                nc.vector.tensor_mul(yts[hp][half:half + 64, :],
                                     py[0:64, :], rb[:])
            # interleave filler work between head pairs
            for fn, fms in HP_FILLERS[qc][hp]:
                qkv_chains(fn, xtiles[fn], fms)
            if qc == 2 and hp == 1:
                out_proj(0, yts_hist[0])
            if qc == 3 and hp == 1:
                out_proj(1, yts_hist[1])
            if qc == 3 and hp == 2:
                out_proj(2, yts_hist[2])
            if hp == 3:
                yts_hist.append(yts)

        # chunk-3 out-proj: chains 6,7 (ki 0-2) into the freed AV banks, then
        # one ki=3 matmul per chain + copy + store.
        op3[6] = avp.tile([128, 512], F32, tag="pyA", name="pyA")[:]
        op3[7] = avp.tile([128, 512], F32, tag="pyB", name="pyB")[:]
        for m in (6, 7):
            for ki in range(3):
                nc.tensor.matmul(op3[m], wost[:, ki, m * 128:(m + 1) * 128],
                                 yts_hist[3][ki][:],
                                 start=(ki == 0), stop=False)
        oo = cop.tile([128, 8, 512], F16, tag="oo", name="oo")
        for m in range(8):
            nc.tensor.matmul(op3[m], wost[:, 3, m * 128:(m + 1) * 128],
                             yts_hist[3][3][:], start=False, stop=True)
            if m % 2 == 0:
                nc.scalar.copy(oo[:, m, :], op3[m])   # ACT is idle by then
            else:
                nc.vector.tensor_copy(oo[:, m, :], op3[m])
                g = m // 2
                nc.sync.dma_start(out=out[3, g], in_=oo[:, g * 2:(g + 1) * 2, :])
    nc.finalize()
    return nc


def _get_nc():
    global _NC_CACHE
    if _NC_CACHE is None:
        _NC_CACHE = _build()
    return _NC_CACHE


def kernel(x, w_qkv, w_out):
    x = np.ascontiguousarray(np.asarray(x), dtype=np.float32)
    w_qkv = np.asarray(w_qkv, dtype=np.float32)
    w_out = np.asarray(w_out, dtype=np.float32)
    nc = _get_nc()

    in_maps = []
    for c in range(NCORES):
        b, j = divmod(c, 2)
        rows = np.r_[j * F:(j + 1) * F,
                     C + j * F:C + (j + 1) * F,
                     2 * C + j * F:2 * C + (j + 1) * F]
        wqkvt = w_qkv[rows, :].T.astype(ml_dtypes.bfloat16)   # [C, 3F]
        wq3 = wqkvt.reshape(KI, 128, 3 * F)
        # q/k m-tiles: wqm[m][p, ki*128+c] = wqkvt[ki*128+p, m*128+c]
        wqm = np.stack([
            np.ascontiguousarray(
                wq3[:, :, m * 128:(m + 1) * 128].transpose(1, 0, 2).reshape(
                    128, KI * 128))
            for m in range(8)])
        wv = np.ascontiguousarray(
            wq3[:, :, 2 * F:3 * F].transpose(1, 0, 2))        # [128, KI, F]
        woutt = w_out[:, j * F:(j + 1) * F].T.astype(ml_dtypes.bfloat16)  # [F, C]
        wot = np.ascontiguousarray(
            woutt.reshape(4, 128, C).transpose(1, 0, 2))      # [128, 4, C]
        # [NT, 128, KI*512]: per chunk, per partition, ki-blocks contiguous
        xT = x[b].T.reshape(KI, 128, NT, 512)
        xr = np.ascontiguousarray(
            xT.transpose(2, 1, 0, 3).reshape(NT, 128, KI * 512)).astype(
                ml_dtypes.bfloat16)
        in_maps.append({"xr": xr, "wqm": wqm, "wv": wv, "wot": wot})

    res = run_bass_kernel_spmd(nc, in_maps, core_ids=list(range(NCORES)))

    def unshard(o):
        # [qc, g, p, m, t] -> [C, T]: feature c = (2g + m)*128 + p
        return np.asarray(o).astype(np.float32).transpose(
            1, 3, 2, 0, 4).reshape(C, T)

    y = np.empty((B, T, C), np.float32)
    for b in range(B):
        y[b] = (unshard(res.results[2 * b]["out"]) +
                unshard(res.results[2 * b + 1]["out"])).T
    return y


# revision 16
# speedup vs baseline: 1.0658x; 1.0314x over previous
"""Causal self-attention (B=4, T=2048, C=1024, H=16) on 8 TRN2 NeuronCores.

Sharding: tensor-parallel pairs. Core c handles batch b = c//2 and head-half
j = c%2 (8 of the 16 heads). Each core computes the QKV projection for its
heads, causal attention, and the out-projection contracted over its half of
the features, producing a partial output. The pair-sum (the "all-reduce after
out_proj" of the tensor-parallel scheme) happens at unshard time on the host.

Structure: one fused loop — the QKV projection for token chunk n+1 is emitted
interleaved with attention for chunk n, so the Tile scheduler fills the
ACT(exp)-bound attention phase with projection matmuls and the PE never idles
long enough for the HAM clock gate to re-throttle. All matmul operands are
bf16; softmax runs in fp32 out of PSUM with the 1/8 scale folded into the ACT
free affine; the causal mask is applied post-exp on GPSIMD (fill=0); the
denominator comes free from a ones-column appended to V so the AV matmul
accumulates sum(exp) in PSUM.

Pipeline details:
- a dummy partition_broadcast at t=0 pre-triggers the GPSIMD custom-op
  library load (otherwise ~9us of mid-kernel stall at the first broadcast);
- input DMAs post on the Sync DGE queue in consumption order (the Scalar DGE
  queue measures ~3x slower startup); x chunk 0 is split so the first chain
  is gated on ~0.75MB; warm-up matmuls on a memset tile spin the PE past the
  HAM clock-gate window while the first inputs stream in;
- each iteration pre-emits the next iteration's first score blocks so the
  exp stream never starves at iteration boundaries;
- the last chunk's out-projection runs ki 0-2 of every chain before the final
  normalize (borrowing the idle scores/AV PSUM banks), so only one matmul per
  chain remains after the last y tile;
- output is stored as fp16 [chunk, pair, p, m, t] (2KB DMA lines), 4 store
  posts per chunk; the host converts/sums in fp32.
"""
import ml_dtypes
import numpy as np
from contextlib import ExitStack

import concourse.bass as bass
from concourse import bacc
import concourse.mybir as mybir
import concourse.tile as tile
from concourse.bass_utils import run_bass_kernel_spmd

B, T, C, H, D = 4, 2048, 1024, 16, 64
NCORES = 8
HPC = H // 2          # heads per core
F = HPC * D           # 512 features per core (per q/k/v)
KI = C // 128         # 8 contraction tiles over C
NT = T // 512         # 4 token chunks
F32 = mybir.dt.float32
F16 = mybir.dt.float16
BF16 = mybir.dt.bfloat16

_NC_CACHE = None


def _build():
    nc = bacc.Bacc("TRN2", target_bir_lowering=False, debug=False)
    # host-reorganized layouts (see kernel()):
    #   xr    [128, KI, T]   x[b].T ki-blocked
    #   wqm   [8, 128, KI*128]  q/k weight m-tiles, ki-blocked
    #   wv    [128, KI, F]   v weights, ki-blocked
    #   wot   [128, 4, C]    out-proj weights, ki-blocked
    xr = nc.dram_tensor("xr", [NT, 128, KI * 512], BF16, kind="ExternalInput").ap()
    wqm = nc.dram_tensor("wqm", [8, 128, KI * 128], BF16, kind="ExternalInput").ap()
    wv = nc.dram_tensor("wv", [128, KI, F], BF16, kind="ExternalInput").ap()
    wot = nc.dram_tensor("wot", [128, 4, C], BF16, kind="ExternalInput").ap()
    # out layout [chunk, m-pair, partition, m, t]: per-partition lines are
    # 2*512 fp16 = 2KB contiguous, which the DMA engines need for full rate
    # (a [C, T] fp16 layout leaves only 1KB lines and measures ~160 GB/s).
    out = nc.dram_tensor("out", [NT, 4, 128, 2, 512], F16,
                         kind="ExternalOutput").ap()

    with ExitStack() as ctx:
        tc = ctx.enter_context(tile.TileContext(nc))

        # persistent SBUF tensors
        qk = ctx.enter_context(tc.tile_pool(name="qk", bufs=1))
        vp = ctx.enter_context(tc.tile_pool(name="vp", bufs=1))
        wqp = ctx.enter_context(tc.tile_pool(name="wqp", bufs=1))
        # qT/kT [128f, T] feature-major (2 heads per tile); vT token-major,
        # 8 head-groups of 65 cols (64 v features + ones col), tail-padded so
        # every 128-col weight window stays in bounds; pad/ones cols only
        # ever feed psum partitions >= 65 which are never read.
        qts = [qk.tile([128, T], BF16, tag=f"q{m}", name=f"q{m}") for m in range(4)]
        kts = [qk.tile([128, T], BF16, tag=f"k{m}", name=f"k{m}") for m in range(4)]
        vts = [vp.tile([128, 583], BF16, tag=f"v{tm}", name=f"v{tm}")
               for tm in range(T // 128)]
        wqmt = wqp.tile([128, 8, KI, 128], BF16, tag="wq", name="wq")
        wvt = wqp.tile([128, KI, F], BF16, tag="wv", name="wv")
        wost = wqp.tile([128, 4, C], BF16, tag="wo", name="wo")

        # working pools
        xp = ctx.enter_context(tc.tile_pool(name="xp", bufs=2))
        pbp = ctx.enter_context(tc.tile_pool(name="pbp", bufs=16))
        yp = ctx.enter_context(tc.tile_pool(name="yp", bufs=4))
        bp = ctx.enter_context(tc.tile_pool(name="bp", bufs=2))
        cop = ctx.enter_context(tc.tile_pool(name="cop", bufs=2))
        # PSUM: 2 banks shared matmul chains (qkv + out-proj), 4 banks scores
        # (double-buffered 2-bank tiles), 2 banks AV accumulators = 8 banks.
        mmp = ctx.enter_context(tc.tile_pool(name="mmp", bufs=2, space="PSUM"))
        scp = ctx.enter_context(tc.tile_pool(name="scp", bufs=2, space="PSUM"))
        avp = ctx.enter_context(tc.tile_pool(name="avp", bufs=1, space="PSUM"))

        fill0 = nc.gpsimd.to_reg(0.0)

        # dummy partition_broadcast: forces the GPSIMD custom-op library
        # (which contains it) to load now, overlapped with the input DMAs,
        # instead of stalling ~9us at the first real broadcast mid-kernel.
        pbw = bp.tile([1, 8], F32, tag="pbw", name="pbw")
        pbo = bp.tile([64, 8], F32, tag="pbo", name="pbo")
        nc.gpsimd.memset(pbw[:], 0.0)
        nc.gpsimd.partition_broadcast(pbo[:, :], pbw[0:1, :])

        def load_x(n):
            t = xp.tile([128, KI, 512], BF16, tag="xct", name="xct")
            nc.sync.dma_start(out=t[:], in_=xr[n].rearrange(
                "p (ki t) -> p ki t", ki=KI))
            return t

        # all input DMAs on the Sync DGE queue, in consumption order; the
        # first chain (m=4, k0) is gated on wqm[4] + the first half of x0.
        nc.sync.dma_start(out=wqmt[:, 4], in_=wqm[4].rearrange(
            "p (ki c) -> p ki c", ki=KI))
        xcs0 = xp.tile([128, KI, 512], BF16, tag="xct", name="xct")
        xsrc0 = xr[0].rearrange("p (ki t) -> p ki t", ki=KI)
        for qtr in range(4):
            nc.sync.dma_start(out=xcs0[:, 2 * qtr:2 * qtr + 2],
                              in_=xsrc0[:, 2 * qtr:2 * qtr + 2])
        nc.sync.dma_start(out=wqmt[:, 0:4], in_=wqm[0:4].rearrange(
            "m p (ki c) -> p m ki c", ki=KI))
        nc.sync.dma_start(out=wqmt[:, 5:8], in_=wqm[5:8].rearrange(
            "m p (ki c) -> p m ki c", ki=KI))
        nc.sync.dma_start(out=wvt[:], in_=wv[:])
        xcs_next = load_x(1)
        nc.sync.dma_start(out=wost[:], in_=wot[:])
        for tm in range(T // 128):
            nc.vector.memset(vts[tm][:], 1.0)

        # HAM warm-up: dummy matmuls on the first memset v tile so the PE
        # clock gate opens while the first inputs are still streaming; sized
        # to bridge until the first real chain's inputs land (~13us).
        wup = scp.tile([128, 2, 512], F32, tag="ps", name="wup")
        for i in range(16):
            nc.tensor.matmul(wup[:, 0, :], vts[0][:, 0:128], vts[0][:, 0:512],
                             start=(i == 0), stop=(i == 15))

        def qkv_chains(n, xct, ms):
            # ms: which of the 12 accumulation chains to emit now
            # (0..7 = q/k feature tiles, 8..11 = v token tiles)
            for m in ms:
                p = mmp.tile([128, 512], F32, tag="mmp", name="mmp")
                if m < 8:
                    for ki in range(KI):
                        nc.tensor.matmul(p[:], wqmt[:, m, ki, :], xct[:, ki, :],
                                         start=(ki == 0), stop=(ki == KI - 1))
                    dst = (qts[m] if m < 4 else kts[m - 4])[:, n * 512:(n + 1) * 512]
                    nc.vector.tensor_copy(dst, p[:])
                else:
                    tmi = m - 8
                    for ki in range(KI):
                        nc.tensor.matmul(p[:],
                                         xct[:, ki, tmi * 128:(tmi + 1) * 128],
                                         wvt[:, ki, :],
                                         start=(ki == 0), stop=(ki == KI - 1))
                    vdst = vts[n * 4 + tmi][:, 0:520].rearrange(
                        "p (h c) -> p h c", c=65)
                    nc.vector.tensor_copy(
                        vdst[:, :, 0:64],
                        p[:].rearrange("p (h c) -> p h c", c=64))

        # chunk 0: k-first chain order so attention(0) unblocks early
        xtiles = {0: xcs0, 1: xcs_next}
        qkv_chains(0, xcs0, [4, 0, 5, 1, 6, 2, 7, 3, 8, 9, 10, 11])

        def sc_block(qc, hp, kt):
            # scores + exp + causal mask for one 128-key block; returns the
            # bf16 probability tile (and the lo offset for the AV matmuls).
            qpair = qts[hp][:, qc * 512:(qc + 1) * 512]
            ksl = kts[hp][:, kt * 128:(kt + 1) * 128]
            d = kt - qc * 4              # diagonal block index
            lo = max(d, 0) * 128         # cols < lo fully masked out
            ps = scp.tile([128, 2, 512], F32, tag="ps", name="ps")
            nc.tensor.matmul(ps[:, 0, lo:512], ksl[0:64, :],
                             qpair[0:64, lo:512],
                             start=True, stop=True, tile_position=(0, 0))
            nc.tensor.matmul(ps[:, 1, lo:512], ksl[64:128, :],
                             qpair[64:128, lo:512],
                             start=True, stop=True, tile_position=(64, 0))
            pb = pbp.tile([128, 2, 512], BF16, tag="pb", name="pb")
            nc.scalar.activation(pb[:, :, lo:512], ps[:, :, lo:512],
                                 mybir.ActivationFunctionType.Exp,
                                 scale=0.125)
            if d >= 0:
                # zero probs where local query j < key partition i
                nc.gpsimd.affine_select(
                    out=pb[:, :, lo:512], in_=pb[:, :, lo:512],
                    compare_op=mybir.AluOpType.is_ge, fill=fill0,
                    base=0, pattern=[[0, 2], [1, 512 - lo]],
                    channel_multiplier=-1)
            return pb, lo

        def out_proj_items(qc_o, yts_o):
            # one boundary work-item per chain; 4 store DMAs per chunk so
            # output streams out as chains finish
            oo = cop.tile([128, 8, 512], F16, tag="oo", name="oo")

            def mk(m):
                def go():
                    po = mmp.tile([128, 512], F32, tag="mmp", name="mmp")[:]
                    for ki in range(4):
                        nc.tensor.matmul(po,
                                         wost[:, ki, m * 128:(m + 1) * 128],
                                         yts_o[ki][:],
                                         start=(ki == 0), stop=(ki == 3))
                    nc.vector.tensor_copy(oo[:, m, :], po)
                    if m % 2 == 1:
                        g = m // 2
                        nc.sync.dma_start(out=out[qc_o, g],
                                          in_=oo[:, g * 2:(g + 1) * 2, :])
                return go
            return [mk(m) for m in range(8)]

        # per-(qc, hp) filler plan: (chunk, chain-ids) of projection work;
        # out-projections for earlier chunks are emitted inside later
        # (ACT-bound) windows — see below
        HP_FILLERS = {
            0: {hp: [(1, [3 * hp, 3 * hp + 1, 3 * hp + 2])] for hp in range(4)},
            1: {hp: [(2, [3 * hp, 3 * hp + 1, 3 * hp + 2])] for hp in range(4)},
            2: {hp: [(3, [3 * hp, 3 * hp + 1, 3 * hp + 2])] for hp in range(4)},
            3: {hp: [] for hp in range(4)},
        }
        iters = [(qc, hp) for qc in range(NT) for hp in range(HPC // 2)]
        pre_pbs = {}
        yts_hist = []
        yts = None
        for idx, (qc, hp) in enumerate(iters):
            n_kt = qc * 4 + 4
            if hp == 0:
                if qc in (0, 1):
                    xtiles[qc + 2] = load_x(qc + 2)
                yts = [yp.tile([128, 512], BF16, tag=f"y{i}", name=f"y{i}")
                       for i in range(4)]
            pyA = avp.tile([128, 512], F32, tag="pyA", name="pyA")
            pyB = avp.tile([128, 512], F32, tag="pyB", name="pyB")
            a0 = 2 * hp * 65
            pbs = pre_pbs.pop((qc, hp), [])
            for kt in range(n_kt):
                if kt < len(pbs):
                    pb, lo = pbs[kt]
                else:
                    pb, lo = sc_block(qc, hp, kt)
                nc.tensor.matmul(pyA[:, lo:512], vts[kt][:, a0:a0 + 128],
                                 pb[:, 0, lo:512],
                                 start=(kt == 0), stop=(kt == n_kt - 1))
                nc.tensor.matmul(pyB[:, lo:512], vts[kt][:, a0 + 65:a0 + 193],
                                 pb[:, 1, lo:512],
                                 start=(kt == 0), stop=(kt == n_kt - 1))
            if (qc, hp) == (3, 3):
                # chunk-3 out-proj prefix: ki 0-2 of chains 0-5 into the now
                # idle mmp + scores banks, before the last normalize gates
                # everything; chains 6,7 take the AV banks right after.
                op3 = [None] * 8
                op3[0] = mmp.tile([128, 512], F32, tag="mmp", name="mmp")[:]
                op3[1] = mmp.tile([128, 512], F32, tag="mmp", name="mmp")[:]
                sct1 = scp.tile([128, 2, 512], F32, tag="ps", name="ps")
                sct2 = scp.tile([128, 2, 512], F32, tag="ps", name="ps")
                op3[2], op3[3] = sct1[:, 0, :], sct1[:, 1, :]
                op3[4], op3[5] = sct2[:, 0, :], sct2[:, 1, :]
                for m in range(6):
                    for ki in range(3):
                        nc.tensor.matmul(op3[m],
                                         wost[:, ki, m * 128:(m + 1) * 128],
                                         yts[ki][:],
                                         start=(ki == 0), stop=False)
            for hh, py in ((0, pyA), (1, pyB)):
                # row 64 of py is sum(exp); normalize y = py[0:64]/py[64]
                # (recip is a custom DVE op: PSUM src reads garbage, so
                # stage the denominator row through SBUF first)
                s1 = bp.tile([1, 512], F32, tag="s1", name="s1")
                nc.vector.tensor_copy(s1[:], py[64:65, :])
                r = bp.tile([1, 512], F32, tag="r", name="r")
                nc.vector.reciprocal_approx_fast(out=r[:], in_=s1[:])
                rb = bp.tile([64, 512], F32, tag="rb", name="rb")
                nc.gpsimd.partition_broadcast(rb[:], r[:])
                half = hh * 64
                nc.vector.tensor_mul(yts[hp][half:half + 64, :],
                                     py[0:64, :], rb[:])
            # boundary work (filler projection chains + out-proj chains) as
            # items, interleaved with pre-emitted score blocks of the next
            # iteration so the PE stream keeps feeding the ACT exp stream.
            # Engine streams execute in order: a monolithic 7-10us burst of
            # projection matmuls here would starve ACT for its duration.
            items, weights = [], []
            for fn, fms in HP_FILLERS[qc][hp]:
                for m in fms:
                    items.append(lambda fn=fn, m=m:
                                 qkv_chains(fn, xtiles[fn], [m]))
                    weights.append(2)
            ops = []
            if qc == 2 and hp == 1:
                ops = out_proj_items(0, yts_hist[0])
            elif qc == 3 and hp == 1:
                ops = out_proj_items(1, yts_hist[1])
            elif qc == 3 and hp == 2:
                ops = out_proj_items(2, yts_hist[2])
            items += ops
            weights += [1] * len(ops)
            if idx + 1 < len(iters):
                nq, nh = iters[idx + 1]
                n_next = nq * 4 + 4
                # interleave only in the ACT-bound regime; earlier phases are
                # PE-bound and pre-block stalls would waste PE time there
                inter = (qc, hp) >= (1, 3)
                pbs2 = []

                def take(k):
                    while k > 0 and len(pbs2) < n_next:
                        pbs2.append(sc_block(nq, nh, len(pbs2)))
                        k -= 1
                take(2 if inter else 3)
                for it, w in zip(items, weights):
                    it()
                    if inter:
                        take(w)
                pre_pbs[(nq, nh)] = pbs2
            else:
                for it in items:
                    it()
            if hp == 3:
                yts_hist.append(yts)

        # chunk-3 out-proj: chains 6,7 (ki 0-2) into the freed AV banks, then
        # one ki=3 matmul per chain + copy + store.
        op3[6] = avp.tile([128, 512], F32, tag="pyA", name="pyA")[:]
        op3[7] = avp.tile([128, 512], F32, tag="pyB", name="pyB")[:]
        for m in (6, 7):
            for ki in range(3):
                nc.tensor.matmul(op3[m], wost[:, ki, m * 128:(m + 1) * 128],
                                 yts_hist[3][ki][:],
                                 start=(ki == 0), stop=False)
        oo = cop.tile([128, 8, 512], F16, tag="oo", name="oo")
        for m in range(8):
            nc.tensor.matmul(op3[m], wost[:, 3, m * 128:(m + 1) * 128],
                             yts_hist[3][3][:], start=False, stop=True)
            if m % 2 == 0:
                nc.scalar.copy(oo[:, m, :], op3[m])   # ACT is idle by then
            else:
                nc.vector.tensor_copy(oo[:, m, :], op3[m])
                g = m // 2
                nc.sync.dma_start(out=out[3, g], in_=oo[:, g * 2:(g + 1) * 2, :])
    nc.finalize()
    return nc


def _get_nc():
    global _NC_CACHE
    if _NC_CACHE is None:
        _NC_CACHE = _build()
    return _NC_CACHE


def kernel(x, w_qkv, w_out):
    x = np.ascontiguousarray(np.asarray(x), dtype=np.float32)
    w_qkv = np.asarray(w_qkv, dtype=np.float32)
    w_out = np.asarray(w_out, dtype=np.float32)
    nc = _get_nc()

    in_maps = []
    for c in range(NCORES):
        b, j = divmod(c, 2)
        rows = np.r_[j * F:(j + 1) * F,
                     C + j * F:C + (j + 1) * F,
                     2 * C + j * F:2 * C + (j + 1) * F]
        wqkvt = w_qkv[rows, :].T.astype(ml_dtypes.bfloat16)   # [C, 3F]
        wq3 = wqkvt.reshape(KI, 128, 3 * F)
        # q/k m-tiles: wqm[m][p, ki*128+c] = wqkvt[ki*128+p, m*128+c]
        wqm = np.stack([
            np.ascontiguousarray(
                wq3[:, :, m * 128:(m + 1) * 128].transpose(1, 0, 2).reshape(
                    128, KI * 128))
            for m in range(8)])
        wv = np.ascontiguousarray(
            wq3[:, :, 2 * F:3 * F].transpose(1, 0, 2))        # [128, KI, F]
        woutt = w_out[:, j * F:(j + 1) * F].T.astype(ml_dtypes.bfloat16)  # [F, C]
        wot = np.ascontiguousarray(
            woutt.reshape(4, 128, C).transpose(1, 0, 2))      # [128, 4, C]
        # [NT, 128, KI*512]: per chunk, per partition, ki-blocks contiguous
        xT = x[b].T.reshape(KI, 128, NT, 512)
        xr = np.ascontiguousarray(
            xT.transpose(2, 1, 0, 3).reshape(NT, 128, KI * 512)).astype(
                ml_dtypes.bfloat16)
        in_maps.append({"xr": xr, "wqm": wqm, "wv": wv, "wot": wot})

    res = run_bass_kernel_spmd(nc, in_maps, core_ids=list(range(NCORES)))

    def unshard(o):
        # [qc, g, p, m, t] -> [C, T]: feature c = (2g + m)*128 + p
        return np.asarray(o).astype(np.float32).transpose(
            1, 3, 2, 0, 4).reshape(C, T)

    y = np.empty((B, T, C), np.float32)
    for b in range(B):
        y[b] = (unshard(res.results[2 * b]["out"]) +
                unshard(res.results[2 * b + 1]["out"])).T
    return y


# revision 19
# speedup vs baseline: 1.1109x; 1.0423x over previous
"""Causal self-attention (B=4, T=2048, C=1024, H=16) on 8 TRN2 NeuronCores.

Sharding: tensor-parallel pairs. Core c handles batch b = c//2 and head-half
j = c%2 (8 of the 16 heads). Each core computes the QKV projection for its
heads, causal attention, and the out-projection contracted over its half of
the features, producing a partial output. The pair-sum (the "all-reduce after
out_proj" of the tensor-parallel scheme) happens at unshard time on the host.

Structure: one fused loop — the QKV projection for token chunk n+1 is emitted
interleaved with attention for chunk n, so the Tile scheduler fills the
ACT(exp)-bound attention phase with projection matmuls and the PE never idles
long enough for the HAM clock gate to re-throttle. All matmul operands are
bf16; softmax runs in fp32 out of PSUM with the 1/8 scale folded into the ACT
free affine; the causal mask is applied post-exp on GPSIMD (fill=0); the
denominator comes free from a ones-column appended to V so the AV matmul
accumulates sum(exp) in PSUM.

Pipeline details:
- a dummy partition_broadcast at t=0 pre-triggers the GPSIMD custom-op
  library load (otherwise ~9us of mid-kernel stall at the first broadcast);
- input DMAs post on the Sync DGE queue in consumption order (the Scalar DGE
  queue measures ~3x slower startup); x chunk 0 is split so the first chain
  is gated on ~0.75MB; warm-up matmuls on a memset tile spin the PE past the
  HAM clock-gate window while the first inputs stream in;
- each iteration pre-emits the next iteration's first score blocks so the
  exp stream never starves at iteration boundaries;
- the last chunk's out-projection runs ki 0-2 of every chain before the final
  normalize (borrowing the idle scores/AV PSUM banks), so only one matmul per
  chain remains after the last y tile;
- output is stored as fp16 [chunk, pair, p, m, t] (2KB DMA lines), 4 store
  posts per chunk; the host converts/sums in fp32.
"""
import ml_dtypes
import numpy as np
from contextlib import ExitStack

import concourse.bass as bass
from concourse import bacc
import concourse.mybir as mybir
import concourse.tile as tile
from concourse.bass_utils import run_bass_kernel_spmd

B, T, C, H, D = 4, 2048, 1024, 16, 64
NCORES = 8
HPC = H // 2          # heads per core
F = HPC * D           # 512 features per core (per q/k/v)
KI = C // 128         # 8 contraction tiles over C
NT = T // 512         # 4 token chunks
F32 = mybir.dt.float32
F16 = mybir.dt.float16
BF16 = mybir.dt.bfloat16

_NC_CACHE = None


def _build():
    nc = bacc.Bacc("TRN2", target_bir_lowering=False, debug=False)
    # host-reorganized layouts (see kernel()):
    #   xr    [128, KI, T]   x[b].T ki-blocked
    #   wqm   [8, 128, KI*128]  q/k weight m-tiles, ki-blocked
    #   wv    [128, KI, F]   v weights, ki-blocked
    #   wot   [128, 4, C]    out-proj weights, ki-blocked
    xr = nc.dram_tensor("xr", [NT, 128, KI * 512], BF16, kind="ExternalInput").ap()
    wqm = nc.dram_tensor("wqm", [8, 128, KI * 128], BF16, kind="ExternalInput").ap()
    wv = nc.dram_tensor("wv", [128, KI, F], BF16, kind="ExternalInput").ap()
    wot = nc.dram_tensor("wot", [128, 4, C], BF16, kind="ExternalInput").ap()
    # out layout [chunk, m-pair, partition, m, t]: per-partition lines are
    # 2*512 fp16 = 2KB contiguous, which the DMA engines need for full rate
    # (a [C, T] fp16 layout leaves only 1KB lines and measures ~160 GB/s).
    out = nc.dram_tensor("out", [NT, 4, 128, 2, 512], F16,
                         kind="ExternalOutput").ap()

    with ExitStack() as ctx:
        tc = ctx.enter_context(tile.TileContext(nc))

        # persistent SBUF tensors
        qk = ctx.enter_context(tc.tile_pool(name="qk", bufs=1))
        vp = ctx.enter_context(tc.tile_pool(name="vp", bufs=1))
        wqp = ctx.enter_context(tc.tile_pool(name="wqp", bufs=1))
        # qT/kT [128f, T] feature-major (2 heads per tile); vT token-major,
        # 8 head-groups of 65 cols (64 v features + ones col), tail-padded so
        # every 128-col weight window stays in bounds; pad/ones cols only
        # ever feed psum partitions >= 65 which are never read.
        qts = [qk.tile([128, T], BF16, tag=f"q{m}", name=f"q{m}") for m in range(4)]
        kts = [qk.tile([128, T], BF16, tag=f"k{m}", name=f"k{m}") for m in range(4)]
        vts = [vp.tile([128, 583], BF16, tag=f"v{tm}", name=f"v{tm}")
               for tm in range(T // 128)]
        wqmt = wqp.tile([128, 8, KI, 128], BF16, tag="wq", name="wq")
        wvt = wqp.tile([128, KI, F], BF16, tag="wv", name="wv")
        wost = wqp.tile([128, 4, C], BF16, tag="wo", name="wo")

        # working pools
        xp = ctx.enter_context(tc.tile_pool(name="xp", bufs=2))
        pbp = ctx.enter_context(tc.tile_pool(name="pbp", bufs=20))
        yp = ctx.enter_context(tc.tile_pool(name="yp", bufs=4))
        bp = ctx.enter_context(tc.tile_pool(name="bp", bufs=2))
        cop = ctx.enter_context(tc.tile_pool(name="cop", bufs=2))
        # PSUM: 2 banks shared matmul chains (qkv + out-proj), 4 banks scores
        # (double-buffered 2-bank tiles), 2 banks AV accumulators = 8 banks.
        mmp = ctx.enter_context(tc.tile_pool(name="mmp", bufs=2, space="PSUM"))
        scp = ctx.enter_context(tc.tile_pool(name="scp", bufs=2, space="PSUM"))
        avp = ctx.enter_context(tc.tile_pool(name="avp", bufs=1, space="PSUM"))

        fill0 = nc.gpsimd.to_reg(0.0)

        # dummy partition_broadcast: forces the GPSIMD custom-op library
        # (which contains it) to load now, overlapped with the input DMAs,
        # instead of stalling ~9us at the first real broadcast mid-kernel.
        pbw = bp.tile([1, 8], F32, tag="pbw", name="pbw")
        pbo = bp.tile([64, 8], F32, tag="pbo", name="pbo")
        nc.gpsimd.memset(pbw[:], 0.0)
        nc.gpsimd.partition_broadcast(pbo[:, :], pbw[0:1, :])

        def load_x(n):
            t = xp.tile([128, KI, 512], BF16, tag="xct", name="xct")
            nc.sync.dma_start(out=t[:], in_=xr[n].rearrange(
                "p (ki t) -> p ki t", ki=KI))
            return t

        # all input DMAs on the Sync DGE queue, in consumption order; the
        # first chain (m=4, k0) is gated on wqm[4] + the first half of x0.
        nc.sync.dma_start(out=wqmt[:, 4], in_=wqm[4].rearrange(
            "p (ki c) -> p ki c", ki=KI))
        xcs0 = xp.tile([128, KI, 512], BF16, tag="xct", name="xct")
        xsrc0 = xr[0].rearrange("p (ki t) -> p ki t", ki=KI)
        for qtr in range(4):
            nc.sync.dma_start(out=xcs0[:, 2 * qtr:2 * qtr + 2],
                              in_=xsrc0[:, 2 * qtr:2 * qtr + 2])
        nc.sync.dma_start(out=wqmt[:, 0:4], in_=wqm[0:4].rearrange(
            "m p (ki c) -> p m ki c", ki=KI))
        nc.sync.dma_start(out=wqmt[:, 5:8], in_=wqm[5:8].rearrange(
            "m p (ki c) -> p m ki c", ki=KI))
        nc.sync.dma_start(out=wvt[:], in_=wv[:])
        xcs_next = load_x(1)
        nc.sync.dma_start(out=wost[:], in_=wot[:])
        for tm in range(T // 128):
            nc.vector.memset(vts[tm][:], 1.0)

        # HAM warm-up: dummy matmuls on the first memset v tile so the PE
        # clock gate opens while the first inputs are still streaming; sized
        # to bridge until the first real chain's inputs land (~13us).
        wup = scp.tile([128, 2, 512], F32, tag="ps", name="wup")
        for i in range(16):
            nc.tensor.matmul(wup[:, 0, :], vts[0][:, 0:128], vts[0][:, 0:512],
                             start=(i == 0), stop=(i == 15))

        def qkv_chains(n, xct, ms):
            # ms: which of the 12 accumulation chains to emit now
            # (0..7 = q/k feature tiles, 8..11 = v token tiles)
            for m in ms:
                p = mmp.tile([128, 512], F32, tag="mmp", name="mmp")
                if m < 8:
                    for ki in range(KI):
                        nc.tensor.matmul(p[:], wqmt[:, m, ki, :], xct[:, ki, :],
                                         start=(ki == 0), stop=(ki == KI - 1))
                    dst = (qts[m] if m < 4 else kts[m - 4])[:, n * 512:(n + 1) * 512]
                    nc.vector.tensor_copy(dst, p[:])
                else:
                    tmi = m - 8
                    for ki in range(KI):
                        nc.tensor.matmul(p[:],
                                         xct[:, ki, tmi * 128:(tmi + 1) * 128],
                                         wvt[:, ki, :],
                                         start=(ki == 0), stop=(ki == KI - 1))
                    vdst = vts[n * 4 + tmi][:, 0:520].rearrange(
                        "p (h c) -> p h c", c=65)
                    nc.vector.tensor_copy(
                        vdst[:, :, 0:64],
                        p[:].rearrange("p (h c) -> p h c", c=64))

        # chunk 0: k-first chain order so attention(0) unblocks early
        xtiles = {0: xcs0, 1: xcs_next}
        qkv_chains(0, xcs0, [4, 0, 5, 1, 6, 2, 7, 3, 8, 9, 10, 11])

        def sc_block(qc, hp, kt):
            # scores + exp + causal mask for one 128-key block; returns the
            # bf16 probability tile (and the lo offset for the AV matmuls).
            qpair = qts[hp][:, qc * 512:(qc + 1) * 512]
            ksl = kts[hp][:, kt * 128:(kt + 1) * 128]
            d = kt - qc * 4              # diagonal block index
            lo = max(d, 0) * 128         # cols < lo fully masked out
            ps = scp.tile([128, 2, 512], F32, tag="ps", name="ps")
            nc.tensor.matmul(ps[:, 0, lo:512], ksl[0:64, :],
                             qpair[0:64, lo:512],
                             start=True, stop=True, tile_position=(0, 0))
            nc.tensor.matmul(ps[:, 1, lo:512], ksl[64:128, :],
                             qpair[64:128, lo:512],
                             start=True, stop=True, tile_position=(64, 0))
            pb = pbp.tile([128, 2, 512], BF16, tag="pb", name="pb")
            nc.scalar.activation(pb[:, :, lo:512], ps[:, :, lo:512],
                                 mybir.ActivationFunctionType.Exp,
                                 scale=0.125)
            if d >= 0:
                # zero probs where local query j < key partition i
                nc.gpsimd.affine_select(
                    out=pb[:, :, lo:512], in_=pb[:, :, lo:512],
                    compare_op=mybir.AluOpType.is_ge, fill=fill0,
                    base=0, pattern=[[0, 2], [1, 512 - lo]],
                    channel_multiplier=-1)
            return pb, lo

        def out_proj_items(qc_o, yts_o):
            # one boundary work-item per chain; 4 store DMAs per chunk so
            # output streams out as chains finish
            oo = cop.tile([128, 8, 512], F16, tag="oo", name="oo")

            def mk(m):
                def go():
                    po = mmp.tile([128, 512], F32, tag="mmp", name="mmp")[:]
                    for ki in range(4):
                        nc.tensor.matmul(po,
                                         wost[:, ki, m * 128:(m + 1) * 128],
                                         yts_o[ki][:],
                                         start=(ki == 0), stop=(ki == 3))
                    nc.vector.tensor_copy(oo[:, m, :], po)
                    if m % 2 == 1:
                        g = m // 2
                        nc.sync.dma_start(out=out[qc_o, g],
                                          in_=oo[:, g * 2:(g + 1) * 2, :])
                return go
            return [mk(m) for m in range(8)]

        # per-(qc, hp) filler plan: (chunk, chain-ids) of projection work;
        # out-projections for earlier chunks are emitted inside later
        # (ACT-bound) windows — see below
        HP_FILLERS = {
            0: {hp: [(1, [3 * hp, 3 * hp + 1, 3 * hp + 2])] for hp in range(4)},
            1: {hp: [(2, [3 * hp, 3 * hp + 1, 3 * hp + 2])] for hp in range(4)},
            2: {hp: [(3, [3 * hp, 3 * hp + 1, 3 * hp + 2])] for hp in range(4)},
            3: {hp: [] for hp in range(4)},
        }
        iters = [(qc, hp) for qc in range(NT) for hp in range(HPC // 2)]
        # Rolling scores/exp cursor: sc_blocks are emitted ("pumped") ahead
        # of their AV consumption — at iteration boundaries interleaved with
        # filler/out-proj items, and inside the kt loop with a +2 lead — so
        # the ACT exp stream never waits on a monolithic PE burst.
        pb_store = {it: {} for it in iters}
        cursor = [0, 0]                      # [iteration index, kt index]

        def pump(n=1):
            while n > 0 and cursor[0] < len(iters):
                cq, ch = iters[cursor[0]]
                pb_store[(cq, ch)][cursor[1]] = sc_block(cq, ch, cursor[1])
                cursor[1] += 1
                if cursor[1] >= cq * 4 + 4:
                    cursor[0] += 1
                    cursor[1] = 0
                n -= 1

        def ensure(it_idx, kt):
            while cursor[0] < it_idx or (cursor[0] == it_idx
                                         and cursor[1] <= kt):
                if cursor[0] >= len(iters):
                    break
                pump(1)

        yts_hist = []
        yts = None
        for idx, (qc, hp) in enumerate(iters):
            n_kt = qc * 4 + 4
            if hp == 0:
                if qc in (0, 1):
                    xtiles[qc + 2] = load_x(qc + 2)
                yts = [yp.tile([128, 512], BF16, tag=f"y{i}", name=f"y{i}")
                       for i in range(4)]
            pyA = avp.tile([128, 512], F32, tag="pyA", name="pyA")
            pyB = avp.tile([128, 512], F32, tag="pyB", name="pyB")
            a0 = 2 * hp * 65
            blocks = pb_store[(qc, hp)]
            for kt in range(n_kt):
                ensure(idx, min(kt + 2, n_kt - 1))
                pb, lo = blocks.pop(kt)
                nc.tensor.matmul(pyA[:, lo:512], vts[kt][:, a0:a0 + 128],
                                 pb[:, 0, lo:512],
                                 start=(kt == 0), stop=(kt == n_kt - 1))
                nc.tensor.matmul(pyB[:, lo:512], vts[kt][:, a0 + 65:a0 + 193],
                                 pb[:, 1, lo:512],
                                 start=(kt == 0), stop=(kt == n_kt - 1))
            if (qc, hp) == (3, 3):
                # chunk-3 out-proj prefix: ki 0-2 of chains 0-5 into the now
                # idle mmp + scores banks, before the last normalize gates
                # everything; chains 6,7 take the AV banks right after.
                op3 = [None] * 8
                op3[0] = mmp.tile([128, 512], F32, tag="mmp", name="mmp")[:]
                op3[1] = mmp.tile([128, 512], F32, tag="mmp", name="mmp")[:]
                sct1 = scp.tile([128, 2, 512], F32, tag="ps", name="ps")
                sct2 = scp.tile([128, 2, 512], F32, tag="ps", name="ps")
                op3[2], op3[3] = sct1[:, 0, :], sct1[:, 1, :]
                op3[4], op3[5] = sct2[:, 0, :], sct2[:, 1, :]
                for m in range(6):
                    for ki in range(3):
                        nc.tensor.matmul(op3[m],
                                         wost[:, ki, m * 128:(m + 1) * 128],
                                         yts[ki][:],
                                         start=(ki == 0), stop=False)
            for hh, py in ((0, pyA), (1, pyB)):
                # row 64 of py is sum(exp); normalize y = py[0:64]/py[64]
                # (recip is a custom DVE op: PSUM src reads garbage, so
                # stage the denominator row through SBUF first)
                s1 = bp.tile([1, 512], F32, tag="s1", name="s1")
                nc.vector.tensor_copy(s1[:], py[64:65, :])
                r = bp.tile([1, 512], F32, tag="r", name="r")
                nc.vector.reciprocal_approx_fast(out=r[:], in_=s1[:])
                rb = bp.tile([64, 512], F32, tag="rb", name="rb")
                nc.gpsimd.partition_broadcast(rb[:], r[:])
                half = hh * 64
                nc.vector.tensor_mul(yts[hp][half:half + 64, :],
                                     py[0:64, :], rb[:])
            # boundary work (filler projection chains + out-proj chains) as
            # items, interleaved with pre-emitted score blocks of the next
            # iteration so the PE stream keeps feeding the ACT exp stream.
            # Engine streams execute in order: a monolithic 7-10us burst of
            # projection matmuls here would starve ACT for its duration.
            items, weights = [], []
            for fn, fms in HP_FILLERS[qc][hp]:
                for m in fms:
                    items.append(lambda fn=fn, m=m:
                                 qkv_chains(fn, xtiles[fn], [m]))
                    weights.append(2)
            ops = []
            if qc == 2 and hp == 1:
                ops = out_proj_items(0, yts_hist[0])
            elif qc == 3 and hp == 1:
                ops = out_proj_items(1, yts_hist[1])
            elif qc == 3 and hp == 2:
                ops = out_proj_items(2, yts_hist[2])
            items += ops
            weights += [1] * len(ops)
            pump(2)
            for it, w in zip(items, weights):
                it()
                pump(w)
            if hp == 3:
                yts_hist.append(yts)

        # chunk-3 out-proj: chains 6,7 (ki 0-2) into the freed AV banks, then
        # one ki=3 matmul per chain + copy + store.
        op3[6] = avp.tile([128, 512], F32, tag="pyA", name="pyA")[:]
        op3[7] = avp.tile([128, 512], F32, tag="pyB", name="pyB")[:]
        for m in (6, 7):
            for ki in range(3):
                nc.tensor.matmul(op3[m], wost[:, ki, m * 128:(m + 1) * 128],
                                 yts_hist[3][ki][:],
                                 start=(ki == 0), stop=False)
        oo = cop.tile([128, 8, 512], F16, tag="oo", name="oo")
        for m in range(8):
            nc.tensor.matmul(op3[m], wost[:, 3, m * 128:(m + 1) * 128],
                             yts_hist[3][3][:], start=False, stop=True)
            if m % 2 == 0:
                nc.scalar.copy(oo[:, m, :], op3[m])   # ACT is idle by then
            else:
                nc.vector.tensor_copy(oo[:, m, :], op3[m])
                g = m // 2
                nc.sync.dma_start(out=out[3, g], in_=oo[:, g * 2:(g + 1) * 2, :])
    nc.finalize()
    return nc


def _get_nc():
    global _NC_CACHE
    if _NC_CACHE is None:
        _NC_CACHE = _build()
    return _NC_CACHE


def kernel(x, w_qkv, w_out):
    x = np.ascontiguousarray(np.asarray(x), dtype=np.float32)
    w_qkv = np.asarray(w_qkv, dtype=np.float32)
    w_out = np.asarray(w_out, dtype=np.float32)
    nc = _get_nc()

    in_maps = []
    for c in range(NCORES):
        b, j = divmod(c, 2)
        rows = np.r_[j * F:(j + 1) * F,
                     C + j * F:C + (j + 1) * F,
                     2 * C + j * F:2 * C + (j + 1) * F]
        wqkvt = w_qkv[rows, :].T.astype(ml_dtypes.bfloat16)   # [C, 3F]
        wq3 = wqkvt.reshape(KI, 128, 3 * F)
        # q/k m-tiles: wqm[m][p, ki*128+c] = wqkvt[ki*128+p, m*128+c]
        wqm = np.stack([
            np.ascontiguousarray(
                wq3[:, :, m * 128:(m + 1) * 128].transpose(1, 0, 2).reshape(
                    128, KI * 128))
            for m in range(8)])
        wv = np.ascontiguousarray(
            wq3[:, :, 2 * F:3 * F].transpose(1, 0, 2))        # [128, KI, F]
        woutt = w_out[:, j * F:(j + 1) * F].T.astype(ml_dtypes.bfloat16)  # [F, C]
        wot = np.ascontiguousarray(
            woutt.reshape(4, 128, C).transpose(1, 0, 2))      # [128, 4, C]
        # [NT, 128, KI*512]: per chunk, per partition, ki-blocks contiguous
        xT = x[b].T.reshape(KI, 128, NT, 512)
        xr = np.ascontiguousarray(
            xT.transpose(2, 1, 0, 3).reshape(NT, 128, KI * 512)).astype(
                ml_dtypes.bfloat16)
        in_maps.append({"xr": xr, "wqm": wqm, "wv": wv, "wot": wot})

    res = run_bass_kernel_spmd(nc, in_maps, core_ids=list(range(NCORES)))

    def unshard(o):
        # [qc, g, p, m, t] -> [C, T]: feature c = (2g + m)*128 + p
        return np.asarray(o).astype(np.float32).transpose(
            1, 3, 2, 0, 4).reshape(C, T)

    y = np.empty((B, T, C), np.float32)
    for b in range(B):
        y[b] = (unshard(res.results[2 * b]["out"]) +
                unshard(res.results[2 * b + 1]["out"])).T
    return y
